# revision 9
# baseline (speedup 1.0000x reference)
"""Trainium2 Bass kernel for nn_CapsuleLayer (B=64, L=512, D=1024, C=32, O=64).

Strategy: data-parallel over batch across 8 NeuronCores (8 batch elements per
core), processed in 2 groups of 4 to fit SBUF. Per core, per batch element:
  u_hatT[co, l] = fc_w.T @ xT (+ fc_b)      PE, fc_w stationary
  u_hat[l, co]  = transpose(u_hatT)          PE transpose
  3 routing iterations, all on-chip:
    c_ij = softmax(b_ij) (no max-sub; logits are O(1))
    s_j  = diag-extract(c_ij.T @ u_hat)      PE cross-matmul, 4 batches packed
                                             into 128 PSUM partitions
    v_j  = squash(s_j)                       ACT/DVE
    b_ij += u_hat . v_j                      PE: 16 accumulating matmuls with
                                             block-diag masked weights vs u_hatT

Everything except batch sharding + input transpose/cast happens on device.
"""

import contextlib
import ctypes
import sys
import types

import numpy as np
import ml_dtypes

B, L, D = 64, 512, 1024
C, O = 32, 64
CO = C * O                  # 2048
ITERS = 3
NCORES = 8
BPC = B // NCORES           # 8 batch elements per core
GB = 4                      # batch elements per routing group (128/C)
NGRP = BPC // GB            # 2
P = 128
KD = D // P                 # 8 contraction chunks
MT = CO // P                # 16 m-tiles of u_hatT (= capsule pairs)
LT = L // P                 # 4 l-chunks
NBANK = CO // 512           # 4 psum banks per s-cross

_BF16 = ml_dtypes.bfloat16

# ---------------------------------------------------------------------------
# NTFF profiling shim (used when tracing is requested by the test harness)
# ---------------------------------------------------------------------------


def _install_ntff_shim():
    if "antenv.axon_hooks" in sys.modules:
        return
    so_path = "/opt/axon/libaxon_pjrt.so"
    hook = None
    try:
        lib = ctypes.CDLL(so_path)
        if hasattr(lib, "axon_start_nrt_profile"):
            lib.axon_start_nrt_profile.argtypes = [
                ctypes.POINTER(ctypes.c_int64),
                ctypes.c_size_t,
            ]
            lib.axon_start_nrt_profile.restype = ctypes.c_int64
            lib.axon_stop_nrt_profile.argtypes = [ctypes.c_char_p]
            lib.axon_stop_nrt_profile.restype = ctypes.c_int64

            @contextlib.contextmanager
            def hook(output_dir, device_ids):
                import jax

                jax.devices()
                if device_ids:
                    ids = (ctypes.c_int64 * len(device_ids))(*device_ids)
                    rc = lib.axon_start_nrt_profile(ids, len(device_ids))
                else:
                    rc = lib.axon_start_nrt_profile(None, 0)
                if rc != 0:
                    raise RuntimeError(f"axon_start_nrt_profile rc={rc}")
                try:
                    yield
                finally:
                    n = lib.axon_stop_nrt_profile(str(output_dir).encode())
                    if n < 0:
                        raise RuntimeError(f"axon_stop_nrt_profile rc={n}")
    except OSError:
        pass
    mod = types.ModuleType("antenv.axon_hooks")
    mod.get_axon_ntff_profile_hook = lambda: hook
    mod.set_axon_ntff_profile_hook = lambda h: None
    sys.modules["antenv.axon_hooks"] = mod

    import concourse.bass_utils as bu

    bu.upload_artifacts = lambda tmpdir: tmpdir


# ---------------------------------------------------------------------------
# Kernel builder
# ---------------------------------------------------------------------------


def build_kernel():
    import concourse.bacc as bacc
    import concourse.tile as tile
    import concourse.mybir as mybir

    f32 = mybir.dt.float32
    bf16 = mybir.dt.bfloat16
    AF = mybir.ActivationFunctionType
    ALU = mybir.AluOpType
    AX = mybir.AxisListType

    nc = bacc.Bacc("TRN2", target_bir_lowering=False, debug=False)

    xt_d = nc.dram_tensor("xt", [BPC, D, L], bf16, kind="ExternalInput")
    w_d = nc.dram_tensor("w", [D, CO], bf16, kind="ExternalInput")
    bias_d = nc.dram_tensor("bias_t", [P, MT], f32, kind="ExternalInput")
    ident_d = nc.dram_tensor("ident", [P, P], bf16, kind="ExternalInput")
    identf_d = nc.dram_tensor("identf", [C, C], f32, kind="ExternalInput")
    identf128_d = nc.dram_tensor("identf128", [P, P], f32, kind="ExternalInput")
    cunif_d = nc.dram_tensor("cunif", [P, C], bf16, kind="ExternalInput")
    m0u_d = nc.dram_tensor("m0u", [O, MT * C], f32, kind="ExternalInput")
    m0l_d = nc.dram_tensor("m0l", [O, MT * C], f32, kind="ExternalInput")
    maskx_d = nc.dram_tensor("mask_x", [P, C], f32, kind="ExternalInput")
    out_d = nc.dram_tensor("v", [BPC * C, O], f32, kind="ExternalOutput")

    with tile.TileContext(nc) as tc, contextlib.ExitStack() as glb:
        const_pool = glb.enter_context(tc.tile_pool(name="consts", bufs=1))
        w_pool = glb.enter_context(tc.tile_pool(name="w", bufs=KD))
        xt_pool = glb.enter_context(tc.tile_pool(name="xt", bufs=2 * KD))
        ut_pool = glb.enter_context(tc.tile_pool(name="ut", bufs=GB * MT))
        u_pool = glb.enter_context(tc.tile_pool(name="u", bufs=GB * LT))
        rt_pool = glb.enter_context(tc.tile_pool(name="rt", bufs=1))
        sm_pool = glb.enter_context(tc.tile_pool(name="sm", bufs=2))

        # --- constants ---
        ident = const_pool.tile([P, P], bf16, name="ident")
        nc.sync.dma_start(ident[:], ident_d[:])
        identf = const_pool.tile([C, C], f32, name="identf")
        nc.sync.dma_start(identf[:], identf_d[:])
        identf128 = const_pool.tile([P, P], f32, name="identf128")
        nc.sync.dma_start(identf128[:], identf128_d[:])
        cunif = const_pool.tile([P, C], bf16, name="cunif")
        nc.sync.dma_start(cunif[:], cunif_d[:])
        m0u = const_pool.tile([O, MT * C], f32, name="m0u")
        nc.sync.dma_start(m0u[:], m0u_d[:])
        m0l = const_pool.tile([O, MT * C], f32, name="m0l")
        nc.sync.dma_start(m0l[:], m0l_d[:])
        maskx = const_pool.tile([P, C], f32, name="maskx")
        nc.sync.dma_start(maskx[:], maskx_d[:])
        bias_sb = const_pool.tile([P, MT], f32, name="bias_sb")
        nc.sync.dma_start(bias_sb[:], bias_d[:])
        eps_sb = const_pool.tile([P, 1], f32, name="eps_sb")
        nc.vector.memset(eps_sb[:], 1e-8)

        w_sb = []
        for k in range(KD):
            wt = w_pool.tile([P, CO], bf16, tag="w", name=f"w{k}")
            nc.sync.dma_start(wt[:], w_d[k * P:(k + 1) * P, :])
            w_sb.append(wt)

        for g in range(NGRP):
            bs = [g * GB + i for i in range(GB)]  # absolute batch ids

            # ---------------- projection + transpose phase ----------------
            UT = {}  # (i, m) -> [P, L] bf16, partitions = co chunk m
            U = {}   # (i, lt) -> [P, CO] bf16, partitions = l chunk lt
            for i in range(GB):
                for lt in range(LT):
                    U[i, lt] = u_pool.tile([P, CO], bf16, tag="u",
                                           name=f"u_g{g}_{i}_{lt}")

            with (
                tc.tile_pool(name=f"ppmm{g}", bufs=4, space="PSUM") as pp_mm,
                tc.tile_pool(name=f"pptr{g}", bufs=4, space="PSUM") as pp_tr,
            ):
                for i, b in enumerate(bs):
                    xt_sb = {}
                    for k in range(KD):
                        t = xt_pool.tile([P, L], bf16, tag="xt",
                                         name=f"xt_g{g}_{i}_{k}")
                        nc.sync.dma_start(t[:], xt_d[b, k * P:(k + 1) * P, :])
                        xt_sb[k] = t

                    for m in range(MT):
                        ps = pp_mm.tile([P, 512], f32, tag="mm",
                                        name=f"ps_g{g}_{m}_{i}")
                        for k in range(KD):
                            nc.tensor.matmul(
                                ps[:],
                                w_sb[k][:, m * P:(m + 1) * P],
                                xt_sb[k][:],
                                start=(k == 0),
                                stop=(k == KD - 1),
                            )
                        ut = ut_pool.tile([P, L], bf16, tag="ut",
                                          name=f"ut_g{g}_{i}_{m}")
                        # u_hatT = psum + bias (bias varies per partition=co)
                        nc.scalar.activation(
                            ut[:], ps[:], AF.Identity,
                            bias=bias_sb[:, m:m + 1],
                        )
                        UT[i, m] = ut
                        # transpose the 4 [128,128] blocks into U layout
                        for lt in range(LT):
                            ptr = pp_tr.tile([P, P], bf16, tag="tr",
                                             name=f"ptr_g{g}_{m}_{i}_{lt}")
                            nc.tensor.transpose(
                                ptr[:], ut[:, lt * P:(lt + 1) * P], ident[:]
                            )
                            nc.vector.tensor_copy(
                                U[i, lt][:, m * P:(m + 1) * P], ptr[:]
                            )

            # ---------------- routing phase ----------------
            b_ij = rt_pool.tile([P, GB, LT, C], f32, tag="bij",
                                name=f"bij_g{g}")
            nc.vector.memset(b_ij[:], 0.0)

            with (
                tc.tile_pool(name=f"pss{g}", bufs=1, space="PSUM") as ps_s_pool,
                tc.tile_pool(name=f"psd{g}", bufs=1, space="PSUM") as ps_d_pool,
                tc.tile_pool(name=f"pst{g}", bufs=2, space="PSUM") as ps_t_pool,
                tc.tile_pool(name=f"psv{g}", bufs=1, space="PSUM") as ps_v_pool,
            ):
                for it in range(ITERS):
                    # --- c_ij ---
                    if it == 0:
                        def c_sl(i, lt):
                            return cunif[:]
                    else:
                        cexp = sm_pool.tile([P, GB * LT, C], f32, tag="cexp", bufs=1,
                                            name=f"cexp_g{g}_{it}")
                        nc.scalar.activation(
                            cexp[:], b_ij[:].rearrange("p b l c -> p (b l) c"),
                            AF.Exp,
                        )
                        csum = sm_pool.tile([P, GB * LT], f32, tag="csum",
                                            name=f"csum_g{g}_{it}")
                        nc.vector.tensor_reduce(csum[:], cexp[:], AX.X, ALU.add)
                        crec = sm_pool.tile([P, GB * LT], f32, tag="crec",
                                            name=f"crec_g{g}_{it}")
                        nc.vector.reciprocal(crec[:], csum[:])
                        c_ij = sm_pool.tile([P, GB * LT, C], bf16, tag="cij",
                                            name=f"cij_g{g}_{it}")
                        nc.vector.tensor_tensor(
                            c_ij[:], cexp[:],
                            crec[:].unsqueeze(2).broadcast_to((P, GB * LT, C)),
                            ALU.mult,
                        )

                        def c_sl(i, lt, c_ij=c_ij):
                            return c_ij[:, i * LT + lt, :]

                    # --- s-pass: cross = c_ij.T @ u_hat, 4 batches packed ---
                    ps_s = ps_s_pool.tile([P, CO], f32, tag="ss",
                                          name=f"pss_g{g}_{it}")
                    for lt in range(LT):
                        for i in range(GB):
                            for n in range(NBANK):
                                nc.tensor.matmul(
                                    ps_s[i * C:(i + 1) * C,
                                         n * 512:(n + 1) * 512],
                                    c_sl(i, lt),
                                    U[i, lt][:, n * 512:(n + 1) * 512],
                                    start=(lt == 0),
                                    stop=(lt == LT - 1),
                                    tile_position=(0, i * C),
                                )

                    # --- extraction: s_all[(i,c), o] = cross[(i,c), (c,o)] ---
                    s_parts = []
                    for n in range(NBANK):
                        tmpb = sm_pool.tile([P, 512], f32, tag="tmpb",
                                            name=f"tmpb_g{g}_{it}_{n}")
                        nc.vector.tensor_tensor(
                            tmpb[:].rearrange("p (c o) -> p c o", c=8),
                            ps_s[:, n * 512:(n + 1) * 512]
                                .rearrange("p (c o) -> p c o", c=8),
                            maskx[:, n * 8:(n + 1) * 8].unsqueeze(2)
                                .broadcast_to((P, 8, O)),
                            ALU.mult,
                        )
                        sp = sm_pool.tile([P, O], f32, tag=f"spart{n}", bufs=1,
                                          name=f"sp_g{g}_{it}_{n}")
                        nc.vector.tensor_reduce(
                            sp[:],
                            tmpb[:].rearrange("p (c o) -> p o c", c=8),
                            AX.X, ALU.add,
                        )
                        s_parts.append(sp)
                    s01 = sm_pool.tile([P, O], f32, tag="s01",
                                       name=f"s01_g{g}_{it}")
                    nc.vector.tensor_tensor(s01[:], s_parts[0][:],
                                            s_parts[1][:], ALU.add)
                    s23 = sm_pool.tile([P, O], f32, tag="s23",
                                       name=f"s23_g{g}_{it}")
                    nc.vector.tensor_tensor(s23[:], s_parts[2][:],
                                            s_parts[3][:], ALU.add)
                    s_all = sm_pool.tile([P, O], f32, tag="sall",
                                         name=f"sall_g{g}_{it}")
                    nc.vector.tensor_tensor(s_all[:], s01[:], s23[:], ALU.add)

                    # --- squash: v = s * sq/(1+sq)/sqrt(sq+1e-8) ---
                    ssq = sm_pool.tile([P, O], f32, tag="ssq", bufs=1,
                                       name=f"ssq_g{g}_{it}")
                    sq = sm_pool.tile([P, 1], f32, tag="sq",
                                      name=f"sq_g{g}_{it}")
                    nc.scalar.activation(ssq[:], s_all[:], AF.Square,
                                         accum_out=sq[:])
                    r1 = sm_pool.tile([P, 1], f32, tag="r1",
                                      name=f"r1_g{g}_{it}")
                    nc.scalar.activation(r1[:], sq[:], AF.Sqrt, bias=eps_sb[:])
                    r2 = sm_pool.tile([P, 1], f32, tag="r2",
                                      name=f"r2_g{g}_{it}")
                    nc.vector.scalar_tensor_tensor(
                        r2[:], sq[:], 1.0, r1[:], ALU.add, ALU.mult,
                    )
                    rr = sm_pool.tile([P, 1], f32, tag="rr",
                                      name=f"rr_g{g}_{it}")
                    nc.vector.reciprocal(rr[:], r2[:])
                    f = sm_pool.tile([P, 1], f32, tag="f",
                                     name=f"f_g{g}_{it}")
                    nc.vector.tensor_tensor(f[:], sq[:], rr[:], ALU.mult)
                    v_all = sm_pool.tile([P, O], f32, tag="vall",
                                         name=f"vall_g{g}_{it}")
                    nc.vector.tensor_scalar_mul(v_all[:], s_all[:], f[:])

                    if it == ITERS - 1:
                        nc.sync.dma_start(
                            out_d[g * P:(g + 1) * P, :], v_all[:]
                        )
                        continue

                    # --- b_ij update ---
                    # vT_all[o, (i,c)] = v_all[(i,c), o]
                    ps_v = ps_v_pool.tile([O, P], f32, tag="vt",
                                          name=f"psv_g{g}_{it}")
                    nc.tensor.transpose(ps_v[:], v_all[:], identf128[:])
                    for i in range(GB):
                        # W[(j,o), (t,c)] = v[2t+j, o] * (c == 2t+j)
                        W = sm_pool.tile([P, MT, C], bf16, tag="W",
                                         name=f"W_g{g}_{it}_{i}")
                        nc.vector.tensor_tensor(
                            W[:O],
                            ps_v[:, i * C:(i + 1) * C:2].unsqueeze(2)
                                .broadcast_to((O, MT, C)),
                            m0u[:].rearrange("p (t c) -> p t c", t=MT),
                            ALU.mult,
                        )
                        nc.vector.tensor_tensor(
                            W[O:],
                            ps_v[:, i * C + 1:(i + 1) * C:2].unsqueeze(2)
                                .broadcast_to((O, MT, C)),
                            m0l[:].rearrange("p (t c) -> p t c", t=MT),
                            ALU.mult,
                        )

                        # deltaT[c, l] = sum_t W_t.T @ u_hatT_t
                        ps_d = ps_d_pool.tile([C, 512], f32, tag="dd",
                                              name=f"psd_g{g}_{it}_{i}")
                        for t in range(MT):
                            nc.tensor.matmul(
                                ps_d[:],
                                W[:, t, :],
                                UT[i, t][:],
                                start=(t == 0),
                                stop=(t == MT - 1),
                            )
                        ds = sm_pool.tile([C, 512], f32, tag="ds", bufs=1,
                                          name=f"ds_g{g}_{it}_{i}")
                        nc.scalar.copy(ds[:], ps_d[:])
                        for lt in range(LT):
                            ps_t = ps_t_pool.tile([P, C], f32, tag="dt",
                                                  name=f"pst_g{g}_{it}_{i}_{lt}")
                            nc.tensor.transpose(
                                ps_t[:], ds[:, lt * P:(lt + 1) * P], identf[:]
                            )
                            nc.vector.tensor_tensor(
                                b_ij[:, i, lt, :], b_ij[:, i, lt, :],
                                ps_t[:], ALU.add,
                            )

    nc.compile()
    return nc


_NC_CACHE = None


def _get_nc():
    global _NC_CACHE
    if _NC_CACHE is None:
        _NC_CACHE = build_kernel()
    return _NC_CACHE


def _make_consts():
    ident = np.eye(P, dtype=_BF16)
    identf = np.eye(C, dtype=np.float32)
    cunif = np.full((P, C), 1.0 / C, dtype=_BF16)
    cc = np.arange(C)
    tt = np.arange(MT)
    m0u = np.broadcast_to(
        (cc[None, :] == 2 * tt[:, None]).astype(np.float32).reshape(1, MT * C),
        (O, MT * C)).copy()
    m0l = np.broadcast_to(
        (cc[None, :] == 2 * tt[:, None] + 1).astype(np.float32).reshape(1, MT * C),
        (O, MT * C)).copy()
    maskx = (cc[None, :] == (np.arange(P) % C)[:, None]).astype(np.float32)
    return ident, identf, cunif, m0u, m0l, maskx


def kernel(inputs, fc_w, fc_b, _trace=False):
    from concourse.bass_utils import run_bass_kernel_spmd

    if _trace:
        _install_ntff_shim()

    nc = _get_nc()

    ident, identf, cunif, m0u, m0l, maskx = _make_consts()
    identf128 = np.eye(P, dtype=np.float32)
    w_bf = np.asarray(fc_w, dtype=np.float32).astype(_BF16)
    bias_t = np.ascontiguousarray(
        np.asarray(fc_b, dtype=np.float32).reshape(MT, P).T
    )
    xt_all = np.ascontiguousarray(
        np.asarray(inputs, dtype=np.float32).transpose(0, 2, 1)
    ).astype(_BF16)

    in_maps = []
    for core in range(NCORES):
        in_maps.append({
            "xt": xt_all[core * BPC:(core + 1) * BPC],
            "w": w_bf,
            "bias_t": bias_t,
            "ident": ident,
            "identf": identf,
            "identf128": identf128,
            "cunif": cunif,
            "m0u": m0u,
            "m0l": m0l,
            "mask_x": maskx,
        })

    res = run_bass_kernel_spmd(
        nc, in_maps, core_ids=list(range(NCORES)), trace=_trace,
    )
    out = np.concatenate(
        [res.results[core]["v"].reshape(BPC, C, O) for core in range(NCORES)],
        axis=0,
    )
    if _trace:
        kernel.last_exec_time_ns = res.exec_time_ns
        kernel.last_results = res
    return out


# revision 10
# speedup vs baseline: 1.0184x; 1.0184x over previous
"""Trainium2 Bass kernel for nn_CapsuleLayer (B=64, L=512, D=1024, C=32, O=64).

Strategy: data-parallel over batch across 8 NeuronCores (8 batch elements per
core), processed in 2 groups of 4 to fit SBUF. Per core, per batch element:
  u_hatT[co, l] = fc_w.T @ xT (+ fc_b)      PE, fc_w stationary
  u_hat[l, co]  = transpose(u_hatT)          PE transpose
  3 routing iterations, all on-chip:
    c_ij = softmax(b_ij) (no max-sub; logits are O(1))
    s_j  = diag-extract(c_ij.T @ u_hat)      PE cross-matmul, 4 batches packed
                                             into 128 PSUM partitions
    v_j  = squash(s_j)                       ACT/DVE
    b_ij += u_hat . v_j                      PE: 16 accumulating matmuls with
                                             block-diag masked weights vs u_hatT

Everything except batch sharding + input transpose/cast happens on device.
"""

import contextlib
import ctypes
import sys
import types

import numpy as np
import ml_dtypes

B, L, D = 64, 512, 1024
C, O = 32, 64
CO = C * O                  # 2048
ITERS = 3
NCORES = 8
BPC = B // NCORES           # 8 batch elements per core
GB = 4                      # batch elements per routing group (128/C)
NGRP = BPC // GB            # 2
P = 128
KD = D // P                 # 8 contraction chunks
MT = CO // P                # 16 m-tiles of u_hatT (= capsule pairs)
LT = L // P                 # 4 l-chunks
NBANK = CO // 512           # 4 psum banks per s-cross

_BF16 = ml_dtypes.bfloat16

# ---------------------------------------------------------------------------
# NTFF profiling shim (used when tracing is requested by the test harness)
# ---------------------------------------------------------------------------


def _install_ntff_shim():
    if "antenv.axon_hooks" in sys.modules:
        return
    so_path = "/opt/axon/libaxon_pjrt.so"
    hook = None
    try:
        lib = ctypes.CDLL(so_path)
        if hasattr(lib, "axon_start_nrt_profile"):
            lib.axon_start_nrt_profile.argtypes = [
                ctypes.POINTER(ctypes.c_int64),
                ctypes.c_size_t,
            ]
            lib.axon_start_nrt_profile.restype = ctypes.c_int64
            lib.axon_stop_nrt_profile.argtypes = [ctypes.c_char_p]
            lib.axon_stop_nrt_profile.restype = ctypes.c_int64

            @contextlib.contextmanager
            def hook(output_dir, device_ids):
                import jax

                jax.devices()
                if device_ids:
                    ids = (ctypes.c_int64 * len(device_ids))(*device_ids)
                    rc = lib.axon_start_nrt_profile(ids, len(device_ids))
                else:
                    rc = lib.axon_start_nrt_profile(None, 0)
                if rc != 0:
                    raise RuntimeError(f"axon_start_nrt_profile rc={rc}")
                try:
                    yield
                finally:
                    n = lib.axon_stop_nrt_profile(str(output_dir).encode())
                    if n < 0:
                        raise RuntimeError(f"axon_stop_nrt_profile rc={n}")
    except OSError:
        pass
    mod = types.ModuleType("antenv.axon_hooks")
    mod.get_axon_ntff_profile_hook = lambda: hook
    mod.set_axon_ntff_profile_hook = lambda h: None
    sys.modules["antenv.axon_hooks"] = mod

    import concourse.bass_utils as bu

    bu.upload_artifacts = lambda tmpdir: tmpdir


# ---------------------------------------------------------------------------
# Kernel builder
# ---------------------------------------------------------------------------


def build_kernel():
    import concourse.bacc as bacc
    import concourse.tile as tile
    import concourse.mybir as mybir

    f32 = mybir.dt.float32
    bf16 = mybir.dt.bfloat16
    AF = mybir.ActivationFunctionType
    ALU = mybir.AluOpType
    AX = mybir.AxisListType

    nc = bacc.Bacc("TRN2", target_bir_lowering=False, debug=False)

    xt_d = nc.dram_tensor("xt", [BPC, D, L], bf16, kind="ExternalInput")
    w_d = nc.dram_tensor("w", [D, CO], bf16, kind="ExternalInput")
    bias_d = nc.dram_tensor("bias_t", [P, MT], f32, kind="ExternalInput")
    ident_d = nc.dram_tensor("ident", [P, P], bf16, kind="ExternalInput")
    identf_d = nc.dram_tensor("identf", [C, C], f32, kind="ExternalInput")
    identf128_d = nc.dram_tensor("identf128", [P, P], f32, kind="ExternalInput")
    cunif_d = nc.dram_tensor("cunif", [P, C], bf16, kind="ExternalInput")
    m0u_d = nc.dram_tensor("m0u", [O, MT * C], f32, kind="ExternalInput")
    m0l_d = nc.dram_tensor("m0l", [O, MT * C], f32, kind="ExternalInput")
    maskx_d = nc.dram_tensor("mask_x", [P, C], f32, kind="ExternalInput")
    out_d = nc.dram_tensor("v", [BPC * C, O], f32, kind="ExternalOutput")

    with tile.TileContext(nc) as tc, contextlib.ExitStack() as glb:
        const_pool = glb.enter_context(tc.tile_pool(name="consts", bufs=1))
        w_pool = glb.enter_context(tc.tile_pool(name="w", bufs=KD))
        xt_pool = glb.enter_context(tc.tile_pool(name="xt", bufs=2 * KD))
        ut_pool = glb.enter_context(tc.tile_pool(name="ut", bufs=GB * MT))
        u_pool = glb.enter_context(tc.tile_pool(name="u", bufs=GB * LT))
        rt_pool = glb.enter_context(tc.tile_pool(name="rt", bufs=1))
        sm_pool = glb.enter_context(tc.tile_pool(name="sm", bufs=2))

        # --- constants ---
        ident = const_pool.tile([P, P], bf16, name="ident")
        nc.sync.dma_start(ident[:], ident_d[:])
        identf = const_pool.tile([C, C], f32, name="identf")
        nc.sync.dma_start(identf[:], identf_d[:])
        identf128 = const_pool.tile([P, P], f32, name="identf128")
        nc.sync.dma_start(identf128[:], identf128_d[:])
        cunif = const_pool.tile([P, C], bf16, name="cunif")
        nc.sync.dma_start(cunif[:], cunif_d[:])
        m0u = const_pool.tile([O, MT * C], f32, name="m0u")
        nc.sync.dma_start(m0u[:], m0u_d[:])
        m0l = const_pool.tile([O, MT * C], f32, name="m0l")
        nc.sync.dma_start(m0l[:], m0l_d[:])
        maskx = const_pool.tile([P, C], f32, name="maskx")
        nc.sync.dma_start(maskx[:], maskx_d[:])
        bias_sb = const_pool.tile([P, MT], f32, name="bias_sb")
        nc.sync.dma_start(bias_sb[:], bias_d[:])
        eps_sb = const_pool.tile([P, 1], f32, name="eps_sb")
        nc.vector.memset(eps_sb[:], 1e-8)

        w_sb = []
        for k in range(KD):
            wt = w_pool.tile([P, CO], bf16, tag="w", name=f"w{k}")
            nc.sync.dma_start(wt[:], w_d[k * P:(k + 1) * P, :])
            w_sb.append(wt)

        for g in range(NGRP):
            bs = [g * GB + i for i in range(GB)]  # absolute batch ids

            # ---------------- projection + transpose phase ----------------
            UT = {}  # (i, m) -> [P, L] bf16, partitions = co chunk m
            U = {}   # (i, lt) -> [P, CO] bf16, partitions = l chunk lt
            for i in range(GB):
                for lt in range(LT):
                    U[i, lt] = u_pool.tile([P, CO], bf16, tag="u",
                                           name=f"u_g{g}_{i}_{lt}")

            with (
                tc.tile_pool(name=f"ppmm{g}", bufs=2, space="PSUM") as pp_mm,
                tc.tile_pool(name=f"pptr{g}", bufs=2, space="PSUM") as pp_tr,
            ):
                for i, b in enumerate(bs):
                    xt_sb = {}
                    for k in range(KD):
                        t = xt_pool.tile([P, L], bf16, tag="xt",
                                         name=f"xt_g{g}_{i}_{k}")
                        nc.scalar.dma_start(t[:], xt_d[b, k * P:(k + 1) * P, :])
                        xt_sb[k] = t

                    for m in range(MT):
                        ps = pp_mm.tile([P, 512], f32, tag="mm",
                                        name=f"ps_g{g}_{m}_{i}")
                        for k in range(KD):
                            nc.tensor.matmul(
                                ps[:],
                                w_sb[k][:, m * P:(m + 1) * P],
                                xt_sb[k][:],
                                start=(k == 0),
                                stop=(k == KD - 1),
                            )
                        ut = ut_pool.tile([P, L], bf16, tag="ut",
                                          name=f"ut_g{g}_{i}_{m}")
                        # u_hatT = psum + bias (bias varies per partition=co)
                        nc.scalar.activation(
                            ut[:], ps[:], AF.Identity,
                            bias=bias_sb[:, m:m + 1],
                        )
                        UT[i, m] = ut
                        # transpose the 4 [128,128] blocks into U layout
                        for lt in range(LT):
                            ptr = pp_tr.tile([P, P], bf16, tag="tr",
                                             name=f"ptr_g{g}_{m}_{i}_{lt}")
                            nc.tensor.transpose(
                                ptr[:], ut[:, lt * P:(lt + 1) * P], ident[:]
                            )
                            nc.vector.tensor_copy(
                                U[i, lt][:, m * P:(m + 1) * P], ptr[:]
                            )

            # ---------------- routing phase ----------------
            b_ij = rt_pool.tile([P, GB, LT, C], f32, tag="bij",
                                name=f"bij_g{g}")
            nc.vector.memset(b_ij[:], 0.0)

            with tc.tile_pool(name=f"pss{g}", bufs=1,
                              space="PSUM") as ps_s_pool:

                def s_pass_and_squash(it, c_get):
                    # bank-major: extraction of bank n overlaps s-pass n+1
                    ps_s = ps_s_pool.tile([P, CO], f32, tag="ss",
                                          name=f"pss_g{g}_{it}")
                    s_parts = []
                    for n in range(NBANK):
                        for lt in range(LT):
                            for i in range(GB):
                                nc.tensor.matmul(
                                    ps_s[i * C:(i + 1) * C,
                                         n * 512:(n + 1) * 512],
                                    c_get(i, lt),
                                    U[i, lt][:, n * 512:(n + 1) * 512],
                                    start=(lt == 0),
                                    stop=(lt == LT - 1),
                                    tile_position=(0, i * C),
                                )
                        tmpb = sm_pool.tile([P, 512], f32, tag="tmpb",
                                            name=f"tmpb_g{g}_{it}_{n}")
                        nc.vector.tensor_tensor(
                            tmpb[:].rearrange("p (c o) -> p c o", c=8),
                            ps_s[:, n * 512:(n + 1) * 512]
                                .rearrange("p (c o) -> p c o", c=8),
                            maskx[:, n * 8:(n + 1) * 8].unsqueeze(2)
                                .broadcast_to((P, 8, O)),
                            ALU.mult,
                        )
                        sp = sm_pool.tile([P, O], f32, tag=f"spart{n}", bufs=1,
                                          name=f"sp_g{g}_{it}_{n}")
                        nc.vector.tensor_reduce(
                            sp[:],
                            tmpb[:].rearrange("p (c o) -> p o c", c=8),
                            AX.X, ALU.add,
                        )
                        s_parts.append(sp)
                    s01 = sm_pool.tile([P, O], f32, tag="s01",
                                       name=f"s01_g{g}_{it}")
                    nc.vector.tensor_tensor(s01[:], s_parts[0][:],
                                            s_parts[1][:], ALU.add)
                    s23 = sm_pool.tile([P, O], f32, tag="s23",
                                       name=f"s23_g{g}_{it}")
                    nc.vector.tensor_tensor(s23[:], s_parts[2][:],
                                            s_parts[3][:], ALU.add)
                    s_all = sm_pool.tile([P, O], f32, tag="sall",
                                         name=f"sall_g{g}_{it}")
                    nc.vector.tensor_tensor(s_all[:], s01[:], s23[:], ALU.add)

                    # squash: v = s * sq/(1+sq)/sqrt(sq+1e-8)
                    ssq = sm_pool.tile([P, O], f32, tag="ssq", bufs=1,
                                       name=f"ssq_g{g}_{it}")
                    sq = sm_pool.tile([P, 1], f32, tag="sq",
                                      name=f"sq_g{g}_{it}")
                    nc.scalar.activation(ssq[:], s_all[:], AF.Square,
                                         accum_out=sq[:])
                    r1 = sm_pool.tile([P, 1], f32, tag="r1",
                                      name=f"r1_g{g}_{it}")
                    nc.scalar.activation(r1[:], sq[:], AF.Sqrt, bias=eps_sb[:])
                    r2 = sm_pool.tile([P, 1], f32, tag="r2",
                                      name=f"r2_g{g}_{it}")
                    nc.vector.scalar_tensor_tensor(
                        r2[:], sq[:], 1.0, r1[:], ALU.add, ALU.mult,
                    )
                    rr = sm_pool.tile([P, 1], f32, tag="rr",
                                      name=f"rr_g{g}_{it}")
                    nc.vector.reciprocal(rr[:], r2[:])
                    f = sm_pool.tile([P, 1], f32, tag="f",
                                     name=f"f_g{g}_{it}")
                    nc.vector.tensor_tensor(f[:], sq[:], rr[:], ALU.mult)
                    v_all = sm_pool.tile([P, O], f32, tag="vall",
                                         name=f"vall_g{g}_{it}")
                    nc.vector.tensor_scalar_mul(v_all[:], s_all[:], f[:])
                    return v_all

                def c_unif_get(i, lt):
                    return cunif[:]

                c_cur = None

                def c_cur_get(i, lt):
                    return c_cur[:, i * LT + lt, :]

                with (
                    tc.tile_pool(name=f"psd{g}", bufs=1,
                                 space="PSUM") as ps_d_pool,
                    tc.tile_pool(name=f"pst{g}", bufs=2,
                                 space="PSUM") as ps_t_pool,
                    tc.tile_pool(name=f"psv{g}", bufs=1,
                                 space="PSUM") as ps_v_pool,
                ):
                    for it in range(ITERS - 1):
                        v_all = s_pass_and_squash(
                            it, c_unif_get if it == 0 else c_cur_get)

                        # vT_all[o, (i,c)] = v_all[(i,c), o]
                        ps_v = ps_v_pool.tile([O, P], f32, tag="vt",
                                              name=f"psv_g{g}_{it}")
                        nc.tensor.transpose(ps_v[:], v_all[:], identf128[:])

                        c_next = sm_pool.tile([P, GB * LT, C], bf16, tag="cij",
                                              name=f"cij_g{g}_{it + 1}")
                        cexp = sm_pool.tile([P, GB * LT, C], f32, tag="cexp",
                                            bufs=1, name=f"cexp_g{g}_{it + 1}")
                        csum = sm_pool.tile([P, GB * LT], f32, tag="csum",
                                            name=f"csum_g{g}_{it + 1}")
                        crec = sm_pool.tile([P, GB * LT], f32, tag="crec",
                                            name=f"crec_g{g}_{it + 1}")

                        for i in range(GB):
                            # W[(j,o), (t,c)] = v[2t+j, o] * (c == 2t+j)
                            W = sm_pool.tile([P, MT, C], bf16, tag="W",
                                             name=f"W_g{g}_{it}_{i}")
                            nc.vector.tensor_tensor(
                                W[:O],
                                ps_v[:, i * C:(i + 1) * C:2].unsqueeze(2)
                                    .broadcast_to((O, MT, C)),
                                m0u[:].rearrange("p (t c) -> p t c", t=MT),
                                ALU.mult,
                            )
                            nc.vector.tensor_tensor(
                                W[O:],
                                ps_v[:, i * C + 1:(i + 1) * C:2].unsqueeze(2)
                                    .broadcast_to((O, MT, C)),
                                m0l[:].rearrange("p (t c) -> p t c", t=MT),
                                ALU.mult,
                            )

                            # deltaT[c, l] = sum_t W_t.T @ u_hatT_t
                            ps_d = ps_d_pool.tile([C, 512], f32, tag="dd",
                                                  name=f"psd_g{g}_{it}_{i}")
                            for t in range(MT):
                                nc.tensor.matmul(
                                    ps_d[:],
                                    W[:, t, :],
                                    UT[i, t][:],
                                    start=(t == 0),
                                    stop=(t == MT - 1),
                                )
                            ds = sm_pool.tile([C, 512], f32, tag="ds", bufs=1,
                                              name=f"ds_g{g}_{it}_{i}")
                            nc.scalar.copy(ds[:], ps_d[:])
                            for lt in range(LT):
                                ps_t = ps_t_pool.tile(
                                    [P, C], f32, tag="dt",
                                    name=f"pst_g{g}_{it}_{i}_{lt}")
                                nc.tensor.transpose(
                                    ps_t[:], ds[:, lt * P:(lt + 1) * P],
                                    identf[:]
                                )
                                nc.vector.tensor_tensor(
                                    b_ij[:, i, lt, :], b_ij[:, i, lt, :],
                                    ps_t[:], ALU.add,
                                )
                            # per-batch softmax for the next iteration
                            nc.scalar.activation(
                                cexp[:, i * LT:(i + 1) * LT, :],
                                b_ij[:, i, :, :], AF.Exp,
                            )
                            nc.vector.tensor_reduce(
                                csum[:, i * LT:(i + 1) * LT],
                                cexp[:, i * LT:(i + 1) * LT, :],
                                AX.X, ALU.add)
                            nc.vector.reciprocal(
                                crec[:, i * LT:(i + 1) * LT],
                                csum[:, i * LT:(i + 1) * LT])
                            nc.vector.tensor_tensor(
                                c_next[:, i * LT:(i + 1) * LT, :],
                                cexp[:, i * LT:(i + 1) * LT, :],
                                crec[:, i * LT:(i + 1) * LT].unsqueeze(2)
                                    .broadcast_to((P, LT, C)),
                                ALU.mult,
                            )
                        c_cur = c_next

                # last iteration: no b_ij update; delta pools are closed so
                # the next group's projection psum can overlap this tail
                v_all = s_pass_and_squash(ITERS - 1, c_cur_get)
                nc.scalar.dma_start(out_d[g * P:(g + 1) * P, :], v_all[:])

    nc.compile()
    return nc


_NC_CACHE = None


def _get_nc():
    global _NC_CACHE
    if _NC_CACHE is None:
        _NC_CACHE = build_kernel()
    return _NC_CACHE


def _make_consts():
    ident = np.eye(P, dtype=_BF16)
    identf = np.eye(C, dtype=np.float32)
    cunif = np.full((P, C), 1.0 / C, dtype=_BF16)
    cc = np.arange(C)
    tt = np.arange(MT)
    m0u = np.broadcast_to(
        (cc[None, :] == 2 * tt[:, None]).astype(np.float32).reshape(1, MT * C),
        (O, MT * C)).copy()
    m0l = np.broadcast_to(
        (cc[None, :] == 2 * tt[:, None] + 1).astype(np.float32).reshape(1, MT * C),
        (O, MT * C)).copy()
    maskx = (cc[None, :] == (np.arange(P) % C)[:, None]).astype(np.float32)
    return ident, identf, cunif, m0u, m0l, maskx


def kernel(inputs, fc_w, fc_b, _trace=False):
    from concourse.bass_utils import run_bass_kernel_spmd

    if _trace:
        _install_ntff_shim()

    nc = _get_nc()

    ident, identf, cunif, m0u, m0l, maskx = _make_consts()
    identf128 = np.eye(P, dtype=np.float32)
    w_bf = np.asarray(fc_w, dtype=np.float32).astype(_BF16)
    bias_t = np.ascontiguousarray(
        np.asarray(fc_b, dtype=np.float32).reshape(MT, P).T
    )
    xt_all = np.ascontiguousarray(
        np.asarray(inputs, dtype=np.float32).transpose(0, 2, 1)
    ).astype(_BF16)

    in_maps = []
    for core in range(NCORES):
        in_maps.append({
            "xt": xt_all[core * BPC:(core + 1) * BPC],
            "w": w_bf,
            "bias_t": bias_t,
            "ident": ident,
            "identf": identf,
            "identf128": identf128,
            "cunif": cunif,
            "m0u": m0u,
            "m0l": m0l,
            "mask_x": maskx,
        })

    res = run_bass_kernel_spmd(
        nc, in_maps, core_ids=list(range(NCORES)), trace=_trace,
    )
    out = np.concatenate(
        [res.results[core]["v"].reshape(BPC, C, O) for core in range(NCORES)],
        axis=0,
    )
    if _trace:
        kernel.last_exec_time_ns = res.exec_time_ns
        kernel.last_results = res
    return out
